# revision 1
# baseline (speedup 1.0000x reference)
"""Trainium2 Bass kernel: multi-head attention (B=4, T=2048, D=2048, H=16).

Sharding: 8 cores = 4 batches x 2 head-groups (tensor-parallel heads, data-
parallel batch). Each core handles one batch and 8 heads (f-slice of 1024
columns of the QKV projections / rows of the out-projection). Host sums the
two partial out-projection results per batch and adds the output bias.

Per-core pipeline (all matmuls bf16 inputs, fp32 PSUM accumulation):
  Phase A: qT[f,t], kT[f,t] (weight-stationary), v[t,f] (x-stationary)
           from xT[d,t] streamed in t-blocks of 512; one weight matrix
           resident at a time (SBUF budget), x re-streamed per pass.
  Phase B: per (head, q-half): S^T[k,q] = kT.T @ qT on PE; exp(scale*S^T)
           on ScalarE -> P^T bf16; PV with ones-augmented V columns gives
           [q, dh | rowsum]; normalize rows by 1/rowsum (DVE reciprocal +
           scale), PE-transpose y -> yT[f,t] (+bv fused, valid since
           softmax rows sum to 1).
  Phase C: out^T[d,t] = WoT.T @ yT accumulated over f-chunks; stored
           transposed, host transposes back.
"""

import sys

if "/opt/trn_rl_repo" not in sys.path:
    sys.path.insert(0, "/opt/trn_rl_repo")

import numpy as np
import ml_dtypes

D = 2048          # d_model
T = 2048          # sequence length
B = 4             # batch
H = 16            # total heads
DH = 128          # head dim
GROUPS = 2        # head groups (tensor-parallel factor per batch)
HG = H // GROUPS  # heads per core = 8
F = HG * DH       # per-core projection width = 1024
P = 128
DC = D // P       # 16 contraction chunks
TC = T // P       # 16 t chunks
NCORES = 8
SCALE = float(1.0 / np.sqrt(DH))

_PROGRAM = None


def _build_program():
    import concourse.bass as bass
    import concourse.tile as tile
    from concourse import bacc, mybir
    from concourse.bass import ts, ds
    from concourse.masks import make_identity

    bf16 = mybir.dt.bfloat16
    f32 = mybir.dt.float32

    nc = bacc.Bacc("TRN2", target_bir_lowering=False, debug=False,
                   num_devices=NCORES)

    xT_d = nc.dram_tensor("xT", [DC, P, T], bf16, kind="ExternalInput")
    wq_d = nc.dram_tensor("wq", [DC, P, F], bf16, kind="ExternalInput")
    wk_d = nc.dram_tensor("wk", [DC, P, F], bf16, kind="ExternalInput")
    wv_d = nc.dram_tensor("wv", [DC, P, F], bf16, kind="ExternalInput")
    wo_d = nc.dram_tensor("wo", [HG, P, D], bf16, kind="ExternalInput")
    bq_d = nc.dram_tensor("bq", [P, HG], f32, kind="ExternalInput")
    bk_d = nc.dram_tensor("bk", [P, HG], f32, kind="ExternalInput")
    bv_d = nc.dram_tensor("bv", [P, HG], f32, kind="ExternalInput")
    out_d = nc.dram_tensor("out", [DC, P, T], f32, kind="ExternalOutput")

    Exp = mybir.ActivationFunctionType.Exp
    Identity = mybir.ActivationFunctionType.Identity

    with tile.TileContext(nc) as tc:
        from contextlib import ExitStack
        with ExitStack() as ctx:
            # ---- persistent pools (allocated first, live whole kernel) ----
            const = ctx.enter_context(tc.tile_pool(name="const", bufs=1))
            qkt = ctx.enter_context(tc.tile_pool(name="qkt", bufs=1))
            vpool = ctx.enter_context(tc.tile_pool(name="vpool", bufs=1))
            ytp = ctx.enter_context(tc.tile_pool(name="ytp", bufs=1))

            ident = const.tile([P, P], bf16, tag="ident")
            make_identity(nc, ident)
            zero_b = const.tile([P, 1], f32, tag="zerob")
            nc.vector.memset(zero_b[:], 0.0)
            bq_sb = const.tile([P, HG], f32, tag="bq")
            bk_sb = const.tile([P, HG], f32, tag="bk")
            bv_sb = const.tile([P, HG], f32, tag="bv")
            nc.sync.dma_start(bq_sb[:], bq_d[:])
            nc.sync.dma_start(bk_sb[:], bk_d[:])
            nc.sync.dma_start(bv_sb[:], bv_d[:])

            qT = [qkt.tile([P, T], bf16, tag=f"qT{h}", name=f"qT{h}")
                  for h in range(HG)]
            kT = [qkt.tile([P, T], bf16, tag=f"kT{h}", name=f"kT{h}")
                  for h in range(HG)]
            v_sb = vpool.tile([P, TC, HG, DH + 1], bf16, tag="v")
            yT = ytp.tile([P, HG, T], bf16, tag="yT")

            # force early allocation of persistent pools (first-use order)
            nc.vector.memset(qT[0][:, 0:1], 0.0)
            nc.vector.memset(v_sb[:, :, :, DH:DH + 1], 1.0)
            nc.vector.memset(yT[:, 0, 0:1], 0.0)

            # ---------------- Phase A: projections ----------------
            with tc.tile_pool(name="wpass", bufs=1) as wpass, \
                 tc.tile_pool(name="xpool", bufs=2) as xpool, \
                 tc.tile_pool(name="ps_proj", bufs=4, space="PSUM") as ps_proj:
                for wd, bias_sb, kind in ((wq_d, bq_sb, "q"),
                                          (wk_d, bk_sb, "k"),
                                          (wv_d, None, "v")):
                    w_sb = wpass.tile([P, DC, F], bf16, tag="w",
                                      name=f"w_{kind}")
                    for dc in range(DC):
                        nc.sync.dma_start(w_sb[:, dc], wd[dc])
                    for tcb in range(4):  # t-blocks of 512
                        xblk = xpool.tile([P, DC, 512], bf16, tag="xblk",
                                          name=f"xblk_{kind}{tcb}")
                        src = xT_d[:, :, ds(tcb * 512, 512)].rearrange(
                            "c p t -> p c t")
                        for dg in range(16):
                            nc.sync.dma_start(xblk[:, ds(dg, 1)],
                                              src[:, ds(dg, 1)])
                        if kind != "v":
                            dst = qT if kind == "q" else kT
                            for h in range(HG):
                                ps = ps_proj.tile([P, 512], f32, tag="ps512",
                                                  name=f"ps_{kind}{tcb}{h}")
                                for dc in range(DC):
                                    nc.tensor.matmul(
                                        ps[:],
                                        w_sb[:, dc, ds(h * DH, DH)],
                                        xblk[:, dc],
                                        start=(dc == 0), stop=(dc == DC - 1))
                                nc.scalar.activation(
                                    dst[h][:, ds(tcb * 512, 512)], ps[:],
                                    Identity, bias=bias_sb[:, ds(h, 1)])
                        else:
                            for tsub in range(4):
                                tc_ = tcb * 4 + tsub
                                psl = ps_proj.tile([P, 512], f32, tag="ps512",
                                                   name=f"psl{tc_}")
                                psr = ps_proj.tile([P, 512], f32, tag="ps512",
                                                   name=f"psr{tc_}")
                                for dc in range(DC):
                                    lhs = xblk[:, dc, ds(tsub * P, P)]
                                    nc.tensor.matmul(
                                        psl[:], lhs, w_sb[:, dc, 0:512],
                                        start=(dc == 0), stop=(dc == DC - 1))
                                    nc.tensor.matmul(
                                        psr[:], lhs, w_sb[:, dc, 512:1024],
                                        start=(dc == 0), stop=(dc == DC - 1))
                                nc.vector.tensor_copy(
                                    v_sb[:, tc_, 0:4, 0:DH],
                                    psl[:].rearrange("p (h d) -> p h d", d=DH))
                                nc.vector.tensor_copy(
                                    v_sb[:, tc_, 4:8, 0:DH],
                                    psr[:].rearrange("p (h d) -> p h d", d=DH))

            # ---------------- Phase B: attention ----------------
            with tc.tile_pool(name="ptpool", bufs=2) as ptpool, \
                 tc.tile_pool(name="ystage", bufs=4) as ystage, \
                 tc.tile_pool(name="rspool", bufs=4) as rspool, \
                 tc.tile_pool(name="ps_st", bufs=2, space="PSUM") as ps_st, \
                 tc.tile_pool(name="ps_pv", bufs=3, space="PSUM") as ps_pv, \
                 tc.tile_pool(name="ps_tr", bufs=1, space="PSUM") as ps_tr:
                for h in range(HG):
                    for half in range(2):
                        q0 = half * (T // 2)
                        pt = ptpool.tile([P, TC, T // 2], bf16, tag="pt",
                                         name=f"pt{h}_{half}")
                        # S^T[k=128, q=1024] per k-chunk; exp -> P^T
                        for kc in range(TC):
                            st = ps_st.tile([P, T // 2], f32, tag="st",
                                            name=f"st{h}{half}{kc}")
                            for qc in range(2):
                                nc.tensor.matmul(
                                    st[:, ds(qc * 512, 512)],
                                    kT[h][:, ds(kc * P, P)],
                                    qT[h][:, ds(q0 + qc * 512, 512)],
                                    start=True, stop=True)
                            nc.scalar.activation(pt[:, kc], st[:], Exp,
                                                 bias=zero_b[:, :],
                                                 scale=SCALE)
                        # PV: out[q=128, dh | rowsum]
                        for qs in range(8):
                            pv = ps_pv.tile([P, DH + 1], f32, tag="pv",
                                            name=f"pv{h}{half}{qs}")
                            for kc in range(TC):
                                nc.tensor.matmul(
                                    pv[:],
                                    pt[:, kc, ds(qs * P, P)],
                                    v_sb[:, kc, h],
                                    start=(kc == 0), stop=(kc == TC - 1))
                            rs = rspool.tile([P, 1], f32, tag="rs",
                                             name=f"rs{h}{half}{qs}")
                            nc.vector.reciprocal(rs[:], pv[:, DH:DH + 1])
                            yst = ystage.tile([P, P], bf16, tag="yst",
                                              name=f"yst{h}{half}{qs}")
                            nc.vector.tensor_scalar_mul(yst[:], pv[:, 0:DH],
                                                        rs[:])
                            tr = ps_tr.tile([P, P], bf16, tag="tr",
                                            name=f"tr{h}{half}{qs}")
                            nc.tensor.transpose(tr[:], yst[:], ident[:])
                            nc.vector.tensor_scalar_add(
                                yT[:, h, ds(q0 + qs * P, P)], tr[:],
                                bv_sb[:, ds(h, 1)])

            # ---------------- Phase C: out-projection ----------------
            with tc.tile_pool(name="wop", bufs=4) as wop, \
                 tc.tile_pool(name="osb", bufs=8) as osb, \
                 tc.tile_pool(name="ps_o", bufs=8, space="PSUM") as ps_o:
                for dch in range(DC):
                    wo_t = wop.tile([P, HG, P], bf16, tag="wo",
                                    name=f"wo{dch}")
                    nc.sync.dma_start(
                        wo_t[:],
                        wo_d[:, :, ds(dch * P, P)].rearrange("h p d -> p h d"))
                    pso = [ps_o.tile([P, 512], f32, tag="pso",
                                     name=f"pso{dch}_{i}") for i in range(4)]
                    for fc in range(HG):
                        for tcb in range(4):
                            nc.tensor.matmul(
                                pso[tcb][:],
                                wo_t[:, fc],
                                yT[:, fc, ds(tcb * 512, 512)],
                                start=(fc == 0), stop=(fc == HG - 1))
                    for tcb in range(4):
                        ot = osb.tile([P, 512], f32, tag="ot",
                                      name=f"ot{dch}_{tcb}")
                        nc.vector.tensor_copy(ot[:], pso[tcb][:])
                        nc.sync.dma_start(out_d[dch, :, ds(tcb * 512, 512)],
                                          ot[:])

    nc.compile()
    return nc


def _get_program():
    global _PROGRAM
    if _PROGRAM is None:
        _PROGRAM = _build_program()
    return _PROGRAM


def _prep_inputs(x, Wq, bq, Wk, bk, Wv, bv, Wo, bo):
    """Build the 8 per-core input maps (host-side sharding, free)."""
    bf = ml_dtypes.bfloat16
    x = np.asarray(x, dtype=np.float32)
    WqT = np.ascontiguousarray(np.asarray(Wq, np.float32).T)  # [D, D]
    WkT = np.ascontiguousarray(np.asarray(Wk, np.float32).T)
    WvT = np.ascontiguousarray(np.asarray(Wv, np.float32).T)
    WoT = np.ascontiguousarray(np.asarray(Wo, np.float32).T)  # [D, D] (f, d)

    in_maps = []
    for c in range(NCORES):
        b, g = divmod(c, GROUPS)
        fsl = slice(g * F, (g + 1) * F)
        xT = np.ascontiguousarray(x[b].T).astype(bf).reshape(DC, P, T)
        m = {
            "xT": xT,
            "wq": np.ascontiguousarray(WqT[:, fsl]).astype(bf).reshape(DC, P, F),
            "wk": np.ascontiguousarray(WkT[:, fsl]).astype(bf).reshape(DC, P, F),
            "wv": np.ascontiguousarray(WvT[:, fsl]).astype(bf).reshape(DC, P, F),
            "wo": np.ascontiguousarray(WoT[fsl, :]).astype(bf).reshape(HG, P, D),
            "bq": np.ascontiguousarray(
                np.asarray(bq, np.float32)[fsl].reshape(HG, P).T),
            "bk": np.ascontiguousarray(
                np.asarray(bk, np.float32)[fsl].reshape(HG, P).T),
            "bv": np.ascontiguousarray(
                np.asarray(bv, np.float32)[fsl].reshape(HG, P).T),
        }
        in_maps.append(m)
    return in_maps


def _combine(results, bo):
    bo = np.asarray(bo, np.float32)
    out = np.empty((B, T, D), dtype=np.float32)
    for b in range(B):
        oT = (results[b * GROUPS]["out"].reshape(D, T).astype(np.float32)
              + results[b * GROUPS + 1]["out"].reshape(D, T).astype(np.float32))
        out[b] = oT.T + bo[None, :]
    return out


def kernel(x, Wq, bq, Wk, bk, Wv, bv, Wo, bo):
    from concourse.bass_utils import run_bass_kernel_spmd

    nc = _get_program()
    in_maps = _prep_inputs(x, Wq, bq, Wk, bk, Wv, bv, Wo, bo)
    res = run_bass_kernel_spmd(nc, in_maps, list(range(NCORES))).results
    return _combine(res, bo)



# revision 5
# speedup vs baseline: 1.0224x; 1.0224x over previous
"""Trainium2 Bass kernel: multi-head attention (B=4, T=2048, D=2048, H=16).

Sharding: 8 cores = 4 batches x 2 head-groups (tensor-parallel heads, data-
parallel batch). Each core handles one batch and 8 heads (f-slice of 1024
columns of the QKV projections / rows of the out-projection). Host sums the
two partial out-projection results per batch and adds the output bias.

v2 structure (single TileContext, phase pools scoped so SBUF regions are
reused and the scheduler can overlap phases):
  q-pass, k-pass, v-pass (weight double-buffered, x streamed per pass),
  then per (head, q-half) units: S^T strip matmuls -> exp on ScalarE ->
  P^T; PV with ones-augmented V (rowsum in col 129); normalize, PE
  transpose, +bv -> yT; finally out-projection accumulated over heads.
  exp starts as soon as the first kT strips exist (subtile deps), pt/yT
  pools reuse the w/x SBUF region after the v-pass frees it.
"""

import sys

if "/opt/trn_rl_repo" not in sys.path:
    sys.path.insert(0, "/opt/trn_rl_repo")

import numpy as np
import ml_dtypes

D = 2048          # d_model
T = 2048          # sequence length
B = 4             # batch
H = 16            # total heads
DH = 128          # head dim
GROUPS = 2        # head groups (tensor-parallel factor per batch)
HG = H // GROUPS  # heads per core = 8
F = HG * DH       # per-core projection width = 1024
P = 128
DC = D // P       # 16 contraction chunks
TC = T // P       # 16 t chunks
NCORES = 8
SCALE = float(1.0 / np.sqrt(DH))

_PROGRAM = None


def _build_program():
    import concourse.bass as bass
    import concourse.tile as tile
    from concourse import bacc, mybir
    from concourse.bass import ts, ds
    from concourse.masks import make_identity

    bf16 = mybir.dt.bfloat16
    f32 = mybir.dt.float32

    nc = bacc.Bacc("TRN2", target_bir_lowering=False, debug=False,
                   num_devices=NCORES)

    xT_d = nc.dram_tensor("xT", [DC, P, T], bf16, kind="ExternalInput")
    wq_d = nc.dram_tensor("wq", [DC, P, F], bf16, kind="ExternalInput")
    wk_d = nc.dram_tensor("wk", [DC, P, F], bf16, kind="ExternalInput")
    wv_d = nc.dram_tensor("wv", [DC, P, F], bf16, kind="ExternalInput")
    wo_d = nc.dram_tensor("wo", [HG, P, D], bf16, kind="ExternalInput")
    bq_d = nc.dram_tensor("bq", [P, HG], f32, kind="ExternalInput")
    bk_d = nc.dram_tensor("bk", [P, HG], f32, kind="ExternalInput")
    bv_d = nc.dram_tensor("bv", [P, HG], f32, kind="ExternalInput")
    out_d = nc.dram_tensor("out", [DC, P, T], f32, kind="ExternalOutput")

    Exp = mybir.ActivationFunctionType.Exp
    Identity = mybir.ActivationFunctionType.Identity

    with tile.TileContext(nc) as tc:
        from contextlib import ExitStack
        with ExitStack() as ctx:
            # ---- persistent pools (allocated first, live whole kernel) ----
            const = ctx.enter_context(tc.tile_pool(name="const", bufs=1))
            qkt = ctx.enter_context(tc.tile_pool(name="qkt", bufs=1))
            vpool = ctx.enter_context(tc.tile_pool(name="vpool", bufs=1))

            ident = const.tile([P, P], bf16, tag="ident")
            make_identity(nc, ident)
            zero_b = const.tile([P, 1], f32, tag="zerob")
            nc.vector.memset(zero_b[:], 0.0)
            bq_sb = const.tile([P, HG], f32, tag="bq")
            bk_sb = const.tile([P, HG], f32, tag="bk")
            bv_sb = const.tile([P, HG], f32, tag="bv")
            nc.sync.dma_start(bq_sb[:], bq_d[:])
            nc.sync.dma_start(bk_sb[:], bk_d[:])
            nc.sync.dma_start(bv_sb[:], bv_d[:])

            qT = [qkt.tile([P, T], bf16, tag=f"qT{h}", name=f"qT{h}")
                  for h in range(HG)]
            kT = [qkt.tile([P, T], bf16, tag=f"kT{h}", name=f"kT{h}")
                  for h in range(HG)]
            v_sb = vpool.tile([P, TC, HG, DH + 1], bf16, tag="v")

            # force early allocation of persistent pools (first-use order)
            nc.vector.memset(qT[0][:, 0:1], 0.0)
            nc.vector.memset(v_sb[:, :, :, DH:DH + 1], 1.0)

            # ---------------- Phase A: projections ----------------
            # w double-buffered: wk/wv DMAs overlap the previous pass.
            with tc.tile_pool(name="wpass", bufs=2) as wpass, \
                 tc.tile_pool(name="xpool", bufs=2) as xpool, \
                 tc.tile_pool(name="ps_proj", bufs=4, space="PSUM") as ps_proj:

                w_tiles = {}
                for wd, kind in ((wq_d, "q"), (wk_d, "k"), (wv_d, "v")):
                    w_sb = wpass.tile([P, DC, F], bf16, tag="w",
                                      name=f"w_{kind}")
                    w_tiles[kind] = w_sb

                def dma_w(kind, wd, lo, hi):
                    # batched: 4 dc-chunks per issue
                    w_sb = w_tiles[kind]
                    src = wd[:, :, :].rearrange("c p f -> p c f")
                    for g in range(lo, hi):
                        nc.sync.dma_start(w_sb[:, ds(4 * g, 4)],
                                          src[:, ds(4 * g, 4)])

                def dma_x(xblk, tcb, lo, hi):
                    src = xT_d[:, :, ds(tcb * 512, 512)].rearrange(
                        "c p t -> p c t")
                    for g in range(lo, hi):
                        nc.sync.dma_start(xblk[:, ds(4 * g, 4)],
                                          src[:, ds(4 * g, 4)])

                # startup: interleave first w chunks with first x chunks so
                # the first matmul's inputs land ASAP.
                dma_w("q", wq_d, 0, 1)
                x0 = xpool.tile([P, DC, 512], bf16, tag="xblk", name="x_q0")
                dma_x(x0, 0, 0, 1)
                dma_w("q", wq_d, 1, 2)
                dma_x(x0, 0, 1, 2)
                dma_w("q", wq_d, 2, 4)
                dma_x(x0, 0, 2, 4)

                for kind in ("q", "k", "v"):
                    w_sb = w_tiles[kind]
                    bias_sb = bq_sb if kind == "q" else bk_sb
                    for tcb in range(4):  # t-blocks of 512
                        if kind == "q" and tcb == 0:
                            xblk = x0
                        else:
                            xblk = xpool.tile([P, DC, 512], bf16, tag="xblk",
                                              name=f"xblk_{kind}{tcb}")
                            dma_x(xblk, tcb, 0, 4)
                        # prefetch next pass's weights mid-pass
                        if tcb == 1:
                            if kind == "q":
                                dma_w("k", wk_d, 0, 4)
                            elif kind == "k":
                                dma_w("v", wv_d, 0, 4)
                        if kind != "v":
                            dst = qT if kind == "q" else kT
                            for h in range(HG):
                                ps = ps_proj.tile([P, 512], f32, tag="ps512",
                                                  name=f"ps_{kind}{tcb}{h}")
                                for dc in range(DC):
                                    nc.tensor.matmul(
                                        ps[:],
                                        w_sb[:, dc, ds(h * DH, DH)],
                                        xblk[:, dc],
                                        start=(dc == 0), stop=(dc == DC - 1))
                                nc.scalar.activation(
                                    dst[h][:, ds(tcb * 512, 512)], ps[:],
                                    Identity, bias=bias_sb[:, ds(h, 1)])
                        else:
                            for tsub in range(4):
                                tc_ = tcb * 4 + tsub
                                psl = ps_proj.tile([P, 512], f32, tag="ps512",
                                                   name=f"psl{tc_}")
                                psr = ps_proj.tile([P, 512], f32, tag="ps512",
                                                   name=f"psr{tc_}")
                                for dc in range(DC):
                                    lhs = xblk[:, dc, ds(tsub * P, P)]
                                    nc.tensor.matmul(
                                        psl[:], lhs, w_sb[:, dc, 0:512],
                                        start=(dc == 0), stop=(dc == DC - 1))
                                    nc.tensor.matmul(
                                        psr[:], lhs, w_sb[:, dc, 512:1024],
                                        start=(dc == 0), stop=(dc == DC - 1))
                                nc.vector.tensor_copy(
                                    v_sb[:, tc_, 0:4, 0:DH],
                                    psl[:].rearrange("p (h d) -> p h d", d=DH))
                                nc.vector.tensor_copy(
                                    v_sb[:, tc_, 4:8, 0:DH],
                                    psr[:].rearrange("p (h d) -> p h d", d=DH))

            # ---------------- Phase B: attention ----------------
            # yT + pt reuse the SBUF region freed by wpass/xpool.
            ytp = ctx.enter_context(tc.tile_pool(name="ytp", bufs=1))
            yT = ytp.tile([P, HG, T], bf16, tag="yT")
            with tc.tile_pool(name="ptpool", bufs=2) as ptpool, \
                 tc.tile_pool(name="ystage", bufs=4) as ystage, \
                 tc.tile_pool(name="rspool", bufs=4) as rspool, \
                 tc.tile_pool(name="ps_st", bufs=2, space="PSUM") as ps_st, \
                 tc.tile_pool(name="ps_pv", bufs=3, space="PSUM") as ps_pv, \
                 tc.tile_pool(name="ps_tr", bufs=1, space="PSUM") as ps_tr:
                for h in range(HG):
                    for half in range(2):
                        q0 = half * (T // 2)
                        pt = ptpool.tile([P, TC, T // 2], bf16, tag="pt",
                                         name=f"pt{h}_{half}")
                        # S^T[k=128, q=1024] per k-chunk; exp -> P^T
                        for kc in range(TC):
                            st = ps_st.tile([P, T // 2], f32, tag="st",
                                            name=f"st{h}{half}{kc}")
                            for qc in range(2):
                                nc.tensor.matmul(
                                    st[:, ds(qc * 512, 512)],
                                    kT[h][:, ds(kc * P, P)],
                                    qT[h][:, ds(q0 + qc * 512, 512)],
                                    start=True, stop=True)
                            nc.scalar.activation(pt[:, kc], st[:], Exp,
                                                 bias=zero_b[:, :],
                                                 scale=SCALE)
                        # PV: out[q=128, dh | rowsum]
                        for qs in range(8):
                            pv = ps_pv.tile([P, DH + 1], f32, tag="pv",
                                            name=f"pv{h}{half}{qs}")
                            for kc in range(TC):
                                nc.tensor.matmul(
                                    pv[:],
                                    pt[:, kc, ds(qs * P, P)],
                                    v_sb[:, kc, h],
                                    start=(kc == 0), stop=(kc == TC - 1))
                            rs = rspool.tile([P, 1], f32, tag="rs",
                                             name=f"rs{h}{half}{qs}")
                            nc.vector.reciprocal(rs[:], pv[:, DH:DH + 1])
                            yst = ystage.tile([P, P], bf16, tag="yst",
                                              name=f"yst{h}{half}{qs}")
                            nc.vector.tensor_scalar_mul(yst[:], pv[:, 0:DH],
                                                        rs[:])
                            tr = ps_tr.tile([P, P], bf16, tag="tr",
                                            name=f"tr{h}{half}{qs}")
                            nc.tensor.transpose(tr[:], yst[:], ident[:])
                            nc.vector.tensor_scalar_add(
                                yT[:, h, ds(q0 + qs * P, P)], tr[:],
                                bv_sb[:, ds(h, 1)])

            # ------------- Phase C: out-projection -------------
            with tc.tile_pool(name="wop", bufs=4) as wop, \
                 tc.tile_pool(name="osb", bufs=8) as osb, \
                 tc.tile_pool(name="ps_o", bufs=8, space="PSUM") as ps_o:
                for dch in range(DC):
                    wo_t = wop.tile([P, HG, P], bf16, tag="wo",
                                    name=f"wo{dch}")
                    nc.sync.dma_start(
                        wo_t[:],
                        wo_d[:, :, ds(dch * P, P)].rearrange(
                            "h p d -> p h d"))
                    pso = [ps_o.tile([P, 512], f32, tag="pso",
                                     name=f"pso{dch}_{i}")
                           for i in range(4)]
                    for fc in range(HG):
                        for tcb in range(4):
                            nc.tensor.matmul(
                                pso[tcb][:],
                                wo_t[:, fc],
                                yT[:, fc, ds(tcb * 512, 512)],
                                start=(fc == 0), stop=(fc == HG - 1))
                    for tcb in range(4):
                        ot = osb.tile([P, 512], f32, tag="ot",
                                      name=f"ot{dch}_{tcb}")
                        nc.vector.tensor_copy(ot[:], pso[tcb][:])
                        nc.sync.dma_start(
                            out_d[dch, :, ds(tcb * 512, 512)], ot[:])

    nc.compile()
    return nc


def _get_program():
    global _PROGRAM
    if _PROGRAM is None:
        _PROGRAM = _build_program()
    return _PROGRAM


def _prep_inputs(x, Wq, bq, Wk, bk, Wv, bv, Wo, bo):
    """Build the 8 per-core input maps (host-side sharding, free)."""
    bf = ml_dtypes.bfloat16
    x = np.asarray(x, dtype=np.float32)
    WqT = np.ascontiguousarray(np.asarray(Wq, np.float32).T)  # [D, D]
    WkT = np.ascontiguousarray(np.asarray(Wk, np.float32).T)
    WvT = np.ascontiguousarray(np.asarray(Wv, np.float32).T)
    WoT = np.ascontiguousarray(np.asarray(Wo, np.float32).T)  # [D, D] (f, d)

    in_maps = []
    for c in range(NCORES):
        b, g = divmod(c, GROUPS)
        fsl = slice(g * F, (g + 1) * F)
        xT = np.ascontiguousarray(x[b].T).astype(bf).reshape(DC, P, T)
        m = {
            "xT": xT,
            "wq": np.ascontiguousarray(WqT[:, fsl]).astype(bf).reshape(DC, P, F),
            "wk": np.ascontiguousarray(WkT[:, fsl]).astype(bf).reshape(DC, P, F),
            "wv": np.ascontiguousarray(WvT[:, fsl]).astype(bf).reshape(DC, P, F),
            "wo": np.ascontiguousarray(WoT[fsl, :]).astype(bf).reshape(HG, P, D),
            "bq": np.ascontiguousarray(
                np.asarray(bq, np.float32)[fsl].reshape(HG, P).T),
            "bk": np.ascontiguousarray(
                np.asarray(bk, np.float32)[fsl].reshape(HG, P).T),
            "bv": np.ascontiguousarray(
                np.asarray(bv, np.float32)[fsl].reshape(HG, P).T),
        }
        in_maps.append(m)
    return in_maps


def _combine(results, bo):
    bo = np.asarray(bo, np.float32)
    out = np.empty((B, T, D), dtype=np.float32)
    for b in range(B):
        oT = (results[b * GROUPS]["out"].reshape(D, T).astype(np.float32)
              + results[b * GROUPS + 1]["out"].reshape(D, T).astype(np.float32))
        out[b] = oT.T + bo[None, :]
    return out


def kernel(x, Wq, bq, Wk, bk, Wv, bv, Wo, bo):
    from concourse.bass_utils import run_bass_kernel_spmd

    nc = _get_program()
    in_maps = _prep_inputs(x, Wq, bq, Wk, bk, Wv, bv, Wo, bo)
    res = run_bass_kernel_spmd(nc, in_maps, list(range(NCORES))).results
    return _combine(res, bo)


# revision 8
# speedup vs baseline: 1.1064x; 1.0821x over previous
"""Trainium2 Bass kernel: multi-head attention (B=4, T=2048, D=2048, H=16).

Sharding: 8 cores = 4 batches x 2 head-groups (tensor-parallel heads, data-
parallel batch). Each core handles one batch and 8 heads (f-slice of 1024
columns of the QKV projections / rows of the out-projection). Host sums the
two partial out-projection results per batch and adds the output bias.

v2 structure (single TileContext, phase pools scoped so SBUF regions are
reused and the scheduler can overlap phases):
  q-pass, k-pass, v-pass (weight double-buffered, x streamed per pass),
  then per (head, q-half) units: S^T strip matmuls -> exp on ScalarE ->
  P^T; PV with ones-augmented V (rowsum in col 129); normalize, PE
  transpose, +bv -> yT; finally out-projection accumulated over heads.
  exp starts as soon as the first kT strips exist (subtile deps), pt/yT
  pools reuse the w/x SBUF region after the v-pass frees it.
"""

import sys

if "/opt/trn_rl_repo" not in sys.path:
    sys.path.insert(0, "/opt/trn_rl_repo")

import numpy as np
import ml_dtypes

D = 2048          # d_model
T = 2048          # sequence length
B = 4             # batch
H = 16            # total heads
DH = 128          # head dim
GROUPS = 2        # head groups (tensor-parallel factor per batch)
HG = H // GROUPS  # heads per core = 8
F = HG * DH       # per-core projection width = 1024
P = 128
DC = D // P       # 16 contraction chunks
TC = T // P       # 16 t chunks
NCORES = 8
SCALE = float(1.0 / np.sqrt(DH))

_PROGRAM = None


def _build_program():
    import concourse.bass as bass
    import concourse.tile as tile
    from concourse import bacc, mybir
    from concourse.bass import ts, ds
    from concourse.masks import make_identity

    bf16 = mybir.dt.bfloat16
    f32 = mybir.dt.float32

    nc = bacc.Bacc("TRN2", target_bir_lowering=False, debug=False,
                   num_devices=NCORES)

    xT_d = nc.dram_tensor("xT", [DC, P, T], bf16, kind="ExternalInput")
    wq_d = nc.dram_tensor("wq", [DC, P, F], bf16, kind="ExternalInput")
    wk_d = nc.dram_tensor("wk", [DC, P, F], bf16, kind="ExternalInput")
    wv_d = nc.dram_tensor("wv", [DC, P, F], bf16, kind="ExternalInput")
    wo_d = nc.dram_tensor("wo", [HG, P, D], bf16, kind="ExternalInput")
    bq_d = nc.dram_tensor("bq", [P, HG], f32, kind="ExternalInput")
    bk_d = nc.dram_tensor("bk", [P, HG], f32, kind="ExternalInput")
    bv_d = nc.dram_tensor("bv", [P, HG], f32, kind="ExternalInput")
    out_d = nc.dram_tensor("out", [DC, P, T], f32, kind="ExternalOutput")

    Exp = mybir.ActivationFunctionType.Exp
    Identity = mybir.ActivationFunctionType.Identity

    with tile.TileContext(nc) as tc:
        from contextlib import ExitStack
        with ExitStack() as ctx:
            # ---- persistent pools (allocated first, live whole kernel) ----
            const = ctx.enter_context(tc.tile_pool(name="const", bufs=1))
            qkt = ctx.enter_context(tc.tile_pool(name="qkt", bufs=1))
            vpool = ctx.enter_context(tc.tile_pool(name="vpool", bufs=1))

            ident = const.tile([P, P], bf16, tag="ident")
            make_identity(nc, ident)
            zero_b = const.tile([P, 1], f32, tag="zerob")
            nc.vector.memset(zero_b[:], 0.0)
            bq_sb = const.tile([P, HG], f32, tag="bq")
            bk_sb = const.tile([P, HG], f32, tag="bk")
            bv_sb = const.tile([P, HG], f32, tag="bv")
            nc.sync.dma_start(bq_sb[:], bq_d[:])
            nc.sync.dma_start(bk_sb[:], bk_d[:])
            nc.sync.dma_start(bv_sb[:], bv_d[:])

            qT = [qkt.tile([P, T], bf16, tag=f"qT{h}", name=f"qT{h}")
                  for h in range(HG)]
            kT = [qkt.tile([P, T], bf16, tag=f"kT{h}", name=f"kT{h}")
                  for h in range(HG)]
            v_sb = vpool.tile([P, TC, HG, DH + 1], bf16, tag="v")

            # force early allocation of persistent pools (first-use order)
            nc.vector.memset(qT[0][:, 0:1], 0.0)
            nc.vector.memset(v_sb[:, :, :, DH:DH + 1], 1.0)

            # ---------------- Phase A: projections ----------------
            # w double-buffered: wk/wv DMAs overlap the previous pass.
            with tc.tile_pool(name="wpass", bufs=2) as wpass, \
                 tc.tile_pool(name="xpool", bufs=2) as xpool, \
                 tc.tile_pool(name="ps_proj", bufs=4, space="PSUM") as ps_proj:

                w_tiles = {}
                for wd, kind in ((wq_d, "q"), (wk_d, "k"), (wv_d, "v")):
                    w_sb = wpass.tile([P, DC, F], bf16, tag="w",
                                      name=f"w_{kind}")
                    w_tiles[kind] = w_sb

                def dma_w(kind, wd, lo, hi):
                    # batched: 4 dc-chunks per issue
                    w_sb = w_tiles[kind]
                    src = wd[:, :, :].rearrange("c p f -> p c f")
                    for g in range(lo, hi):
                        nc.sync.dma_start(w_sb[:, ds(4 * g, 4)],
                                          src[:, ds(4 * g, 4)])

                def dma_x(xblk, tcb, lo, hi):
                    src = xT_d[:, :, ds(tcb * 512, 512)].rearrange(
                        "c p t -> p c t")
                    for g in range(lo, hi):
                        nc.sync.dma_start(xblk[:, ds(4 * g, 4)],
                                          src[:, ds(4 * g, 4)])

                # startup: interleave first w chunks with first x chunks at
                # fine grain so the first matmul's inputs land ASAP.
                x0 = xpool.tile([P, DC, 512], bf16, tag="xblk", name="x_q0")
                w_q = w_tiles["q"]
                src_w = wq_d[:, :, :].rearrange("c p f -> p c f")
                src_x = xT_d[:, :, ds(0, 512)].rearrange("c p t -> p c t")
                for dc in range(4):
                    nc.sync.dma_start(w_q[:, ds(dc, 1)], src_w[:, ds(dc, 1)])
                    nc.sync.dma_start(x0[:, ds(dc, 1)], src_x[:, ds(dc, 1)])
                for g in range(1, 4):
                    nc.sync.dma_start(w_q[:, ds(4 * g, 4)],
                                      src_w[:, ds(4 * g, 4)])
                    nc.sync.dma_start(x0[:, ds(4 * g, 4)],
                                      src_x[:, ds(4 * g, 4)])

                for kind in ("q", "k", "v"):
                    w_sb = w_tiles[kind]
                    bias_sb = bq_sb if kind == "q" else bk_sb
                    for tcb in range(4):  # t-blocks of 512
                        if kind == "q" and tcb == 0:
                            xblk = x0
                        else:
                            xblk = xpool.tile([P, DC, 512], bf16, tag="xblk",
                                              name=f"xblk_{kind}{tcb}")
                            dma_x(xblk, tcb, 0, 4)
                        # prefetch next pass's weights mid-pass
                        if tcb == 1:
                            if kind == "q":
                                dma_w("k", wk_d, 0, 4)
                            elif kind == "k":
                                dma_w("v", wv_d, 0, 4)
                        if kind != "v":
                            dst = qT if kind == "q" else kT
                            for h in range(HG):
                                ps = ps_proj.tile([P, 512], f32, tag="ps512",
                                                  name=f"ps_{kind}{tcb}{h}")
                                for dc in range(DC):
                                    nc.tensor.matmul(
                                        ps[:],
                                        w_sb[:, dc, ds(h * DH, DH)],
                                        xblk[:, dc],
                                        start=(dc == 0), stop=(dc == DC - 1))
                                nc.scalar.activation(
                                    dst[h][:, ds(tcb * 512, 512)], ps[:],
                                    Identity, bias=bias_sb[:, ds(h, 1)])
                        else:
                            for tsub in range(4):
                                tc_ = tcb * 4 + tsub
                                psl = ps_proj.tile([P, 512], f32, tag="ps512",
                                                   name=f"psl{tc_}")
                                psr = ps_proj.tile([P, 512], f32, tag="ps512",
                                                   name=f"psr{tc_}")
                                for dc in range(DC):
                                    lhs = xblk[:, dc, ds(tsub * P, P)]
                                    nc.tensor.matmul(
                                        psl[:], lhs, w_sb[:, dc, 0:512],
                                        start=(dc == 0), stop=(dc == DC - 1))
                                    nc.tensor.matmul(
                                        psr[:], lhs, w_sb[:, dc, 512:1024],
                                        start=(dc == 0), stop=(dc == DC - 1))
                                nc.vector.tensor_copy(
                                    v_sb[:, tc_, 0:4, 0:DH],
                                    psl[:].rearrange("p (h d) -> p h d", d=DH))
                                nc.vector.tensor_copy(
                                    v_sb[:, tc_, 4:8, 0:DH],
                                    psr[:].rearrange("p (h d) -> p h d", d=DH))

            # ---------------- Phase B: attention ----------------
            # yT + pt reuse the SBUF region freed by wpass/xpool.
            ytp = ctx.enter_context(tc.tile_pool(name="ytp", bufs=1))
            yT = ytp.tile([P, HG, T], bf16, tag="yT")
            with tc.tile_pool(name="ptpool", bufs=2) as ptpool, \
                 tc.tile_pool(name="ystage", bufs=4) as ystage, \
                 tc.tile_pool(name="rspool", bufs=4) as rspool, \
                 tc.tile_pool(name="ps_st", bufs=2, space="PSUM") as ps_st, \
                 tc.tile_pool(name="ps_pv", bufs=3, space="PSUM") as ps_pv, \
                 tc.tile_pool(name="ps_tr", bufs=1, space="PSUM") as ps_tr:

                def emit_scores(h, half):
                    # S^T[k=128, q=1024] strips; exp -> P^T
                    q0 = half * (T // 2)
                    pt = ptpool.tile([P, TC, T // 2], bf16, tag="pt",
                                     name=f"pt{h}_{half}")
                    for kc in range(TC):
                        st = ps_st.tile([P, T // 2], f32, tag="st",
                                        name=f"st{h}{half}{kc}")
                        for qc in range(2):
                            nc.tensor.matmul(
                                st[:, ds(qc * 512, 512)],
                                kT[h][:, ds(kc * P, P)],
                                qT[h][:, ds(q0 + qc * 512, 512)],
                                start=True, stop=True)
                        nc.scalar.activation(pt[:, kc], st[:], Exp,
                                             bias=zero_b[:, :],
                                             scale=SCALE)
                    return pt

                def emit_pv(h, half, pt):
                    # PV: out[q=128, dh | rowsum]; normalize; transpose -> yT
                    q0 = half * (T // 2)
                    for qs in range(8):
                        pv = ps_pv.tile([P, DH + 1], f32, tag="pv",
                                        name=f"pv{h}{half}{qs}")
                        for kc in range(TC):
                            nc.tensor.matmul(
                                pv[:],
                                pt[:, kc, ds(qs * P, P)],
                                v_sb[:, kc, h],
                                start=(kc == 0), stop=(kc == TC - 1))
                        rs = rspool.tile([P, 1], f32, tag="rs",
                                         name=f"rs{h}{half}{qs}")
                        nc.vector.reciprocal(rs[:], pv[:, DH:DH + 1])
                        yst = ystage.tile([P, P], bf16, tag="yst",
                                          name=f"yst{h}{half}{qs}")
                        nc.vector.tensor_scalar_mul(yst[:], pv[:, 0:DH],
                                                    rs[:])
                        tr = ps_tr.tile([P, P], bf16, tag="tr",
                                        name=f"tr{h}{half}{qs}")
                        nc.tensor.transpose(tr[:], yst[:], ident[:])
                        nc.vector.tensor_scalar_add(
                            yT[:, h, ds(q0 + qs * P, P)], tr[:],
                            bv_sb[:, ds(h, 1)])

                # software pipeline: emit scores(u+1) before pv(u) so the
                # PE queue never blocks on exp(u) finishing.
                units = [(h, half) for h in range(HG) for half in range(2)]
                prev = None
                for h, half in units:
                    pt = emit_scores(h, half)
                    if prev is not None:
                        emit_pv(prev[0], prev[1], prev[2])
                    prev = (h, half, pt)
                emit_pv(prev[0], prev[1], prev[2])

            # ------------- Phase C: out-projection -------------
            with tc.tile_pool(name="wop", bufs=4) as wop, \
                 tc.tile_pool(name="osb", bufs=8) as osb, \
                 tc.tile_pool(name="ps_o", bufs=8, space="PSUM") as ps_o:
                for dch in range(DC):
                    wo_t = wop.tile([P, HG, P], bf16, tag="wo",
                                    name=f"wo{dch}")
                    nc.sync.dma_start(
                        wo_t[:],
                        wo_d[:, :, ds(dch * P, P)].rearrange(
                            "h p d -> p h d"))
                    pso = [ps_o.tile([P, 512], f32, tag="pso",
                                     name=f"pso{dch}_{i}")
                           for i in range(4)]
                    for fc in range(HG):
                        for tcb in range(4):
                            nc.tensor.matmul(
                                pso[tcb][:],
                                wo_t[:, fc],
                                yT[:, fc, ds(tcb * 512, 512)],
                                start=(fc == 0), stop=(fc == HG - 1))
                    for tcb in range(4):
                        ot = osb.tile([P, 512], f32, tag="ot",
                                      name=f"ot{dch}_{tcb}")
                        nc.vector.tensor_copy(ot[:], pso[tcb][:])
                        nc.sync.dma_start(
                            out_d[dch, :, ds(tcb * 512, 512)], ot[:])

    nc.compile()
    return nc


def _get_program():
    global _PROGRAM
    if _PROGRAM is None:
        _PROGRAM = _build_program()
    return _PROGRAM


def _prep_inputs(x, Wq, bq, Wk, bk, Wv, bv, Wo, bo):
    """Build the 8 per-core input maps (host-side sharding, free)."""
    bf = ml_dtypes.bfloat16
    x = np.asarray(x, dtype=np.float32)
    WqT = np.ascontiguousarray(np.asarray(Wq, np.float32).T)  # [D, D]
    WkT = np.ascontiguousarray(np.asarray(Wk, np.float32).T)
    WvT = np.ascontiguousarray(np.asarray(Wv, np.float32).T)
    WoT = np.ascontiguousarray(np.asarray(Wo, np.float32).T)  # [D, D] (f, d)

    in_maps = []
    for c in range(NCORES):
        b, g = divmod(c, GROUPS)
        fsl = slice(g * F, (g + 1) * F)
        xT = np.ascontiguousarray(x[b].T).astype(bf).reshape(DC, P, T)
        m = {
            "xT": xT,
            "wq": np.ascontiguousarray(WqT[:, fsl]).astype(bf).reshape(DC, P, F),
            "wk": np.ascontiguousarray(WkT[:, fsl]).astype(bf).reshape(DC, P, F),
            "wv": np.ascontiguousarray(WvT[:, fsl]).astype(bf).reshape(DC, P, F),
            "wo": np.ascontiguousarray(WoT[fsl, :]).astype(bf).reshape(HG, P, D),
            "bq": np.ascontiguousarray(
                np.asarray(bq, np.float32)[fsl].reshape(HG, P).T),
            "bk": np.ascontiguousarray(
                np.asarray(bk, np.float32)[fsl].reshape(HG, P).T),
            "bv": np.ascontiguousarray(
                np.asarray(bv, np.float32)[fsl].reshape(HG, P).T),
        }
        in_maps.append(m)
    return in_maps


def _combine(results, bo):
    bo = np.asarray(bo, np.float32)
    out = np.empty((B, T, D), dtype=np.float32)
    for b in range(B):
        oT = (results[b * GROUPS]["out"].reshape(D, T).astype(np.float32)
              + results[b * GROUPS + 1]["out"].reshape(D, T).astype(np.float32))
        out[b] = oT.T + bo[None, :]
    return out


def kernel(x, Wq, bq, Wk, bk, Wv, bv, Wo, bo):
    from concourse.bass_utils import run_bass_kernel_spmd

    nc = _get_program()
    in_maps = _prep_inputs(x, Wq, bq, Wk, bk, Wv, bv, Wo, bo)
    res = run_bass_kernel_spmd(nc, in_maps, list(range(NCORES))).results
    return _combine(res, bo)
